# revision 8
# baseline (speedup 1.0000x reference)
"""DFT-D3 dispersion energy kernel for 8 Trainium2 NeuronCores.

Strategy (per sharding hint): shard the 1.6M-edge list across 8 cores
(200k edges each), replicate atoms/tables. Two device launches:

  Launch 1 (CN): edges sorted by i-atom on host into a padded
    [50048, K] slot matrix per core; device computes the D3
    coordination-number counting function per slot, dense-reduces rows
    to per-atom CN partials, AllReduce-psums CN across the 8 cores, and
    computes the per-atom Gaussian C6-interpolation weights W[50048,5].

  Host: gathers W rows to edge endpoints (index marshalling only).

  Launch 2 (energy): plain per-edge arrays; device computes BJ-damped
    pair energies e = c6_ij * u(d) with c6_ij = Wi^T B Wj (B = gathered
    5x5 C6 block), reduces to per-core partials; host sums partials.

All transcendentals use the {Ln, Exp} activation table set only
(sigmoid via exp, sqrt via exp(0.5 ln)) so there is a single ACT table
load in the whole kernel.
"""

import sys

sys.path.insert(0, "/opt/trn_rl_repo")

import numpy as np

import concourse.bacc as bacc
import concourse.bass as bass
import concourse.mybir as mybir
import concourse.tile as tile
from concourse import bass_utils

F32 = mybir.dt.float32
AX = mybir.AluOpType
ACTF = mybir.ActivationFunctionType

# Our only transcendentals are Ln and Exp. Steer the ACT table-load pass
# to the combined natural_log_exp set so the kernel needs exactly one
# table load instead of thrashing between the ln-only and exp-only sets
# (~2.7us per reload).
_orig_get_tables = bacc.get_activation_tables


def _ln_exp_tables(module_arch):
    tables = dict(_orig_get_tables(module_arch))
    out = {}
    for name, funcs in tables.items():
        if name == "natural_log_exp_and_others":
            out[name] = funcs
        else:
            out[name] = funcs - {ACTF.Ln, ACTF.Exp}
    return out


bacc.get_activation_tables = _ln_exp_tables

# D3 constants
K1 = 16.0
K2 = 4.0 / 3.0
K3 = 4.0
A1, A2, S6, S8 = 0.4, 5.0, 1.0, 0.78
CN_CUTOFF2 = 25.0 * 25.0
DISP_CUTOFF2 = 50.0 * 50.0

N_ATOMS = 50000
NP_ATOMS = 50048  # = 128 * 391
GRID_C = 391
N_EDGES = 1_600_000
N_CORES = 8
E_CORE = N_EDGES // N_CORES  # 200000
NREF = 5

# launch-2 chunking: slots per partition per chunk
L2_C = 200
L2_NCH = 8  # 128*200*8 = 204800 >= 200000
E_PAD2 = 128 * L2_C * L2_NCH

_cache = {}


def _runner(nc, out_names):
    """Compile once, return a callable(in_maps) -> list of out dicts."""
    import jax
    from jax.sharding import Mesh, PartitionSpec
    from jax.experimental.shard_map import shard_map
    from concourse import bass2jax

    bass2jax.install_neuronx_cc_hook()

    partition_name = (
        nc.partition_id_tensor.name if nc.partition_id_tensor else None
    )
    in_names = []
    out_avals = []
    zero_outs = []
    onames = []
    for alloc in nc.m.functions[0].allocations:
        if not isinstance(alloc, mybir.MemoryLocationSet):
            continue
        name = alloc.memorylocations[0].name
        if alloc.kind == "ExternalInput":
            if name != partition_name:
                in_names.append(name)
        elif alloc.kind == "ExternalOutput":
            shape = list(alloc.tensor_shape)
            dt = mybir.dt.np(alloc.dtype)
            onames.append(name)
            out_avals.append(jax.core.ShapedArray(shape, dt))
            zero_outs.append(np.zeros(shape, dt))
    n_params = len(in_names)
    all_in = list(in_names) + list(onames)
    if partition_name is not None:
        all_in.append(partition_name)

    from concourse.bass2jax import _bass_exec_p, partition_id_tensor

    def _body(*args):
        operands = list(args)
        if partition_name is not None:
            operands.append(partition_id_tensor())
        outs = _bass_exec_p.bind(
            *operands,
            out_avals=tuple(out_avals),
            in_names=tuple(all_in),
            out_names=tuple(onames),
            lowering_input_output_aliases=(),
            sim_require_finite=True,
            sim_require_nnan=True,
            nc=nc,
        )
        return tuple(outs)

    devices = jax.devices()[:N_CORES]
    mesh = Mesh(np.asarray(devices), ("core",))
    donate = tuple(range(n_params, n_params + len(onames)))
    sharded = jax.jit(
        shard_map(
            _body,
            mesh=mesh,
            in_specs=(PartitionSpec("core"),) * (n_params + len(onames)),
            out_specs=(PartitionSpec("core"),) * len(onames),
            check_rep=False,
        ),
        donate_argnums=donate,
        keep_unused=True,
    )

    def _concat(in_maps):
        per_core = [[np.asarray(m[n]) for n in in_names] for m in in_maps]
        return [
            np.concatenate([per_core[c][i] for c in range(N_CORES)], axis=0)
            for i in range(n_params)
        ]

    def _zeros():
        return [
            np.zeros((N_CORES * z.shape[0], *z.shape[1:]), z.dtype)
            for z in zero_outs
        ]

    def _unpack(out_arrs):
        return [
            {
                n: np.asarray(out_arrs[i]).reshape(
                    N_CORES, *out_avals[i].shape
                )[c]
                for i, n in enumerate(onames)
            }
            for c in range(N_CORES)
        ]

    def run(in_maps):
        return _unpack(sharded(*_concat(in_maps), *_zeros()))

    def run_timed(in_maps, iters=3):
        """Pre-stage inputs on device, time execute-only. Returns
        (results, best_seconds)."""
        import time
        from jax.sharding import NamedSharding

        sh = NamedSharding(mesh, PartitionSpec("core"))
        staged = [jax.device_put(a, sh) for a in _concat(in_maps)]
        out = sharded(*staged, *_zeros())  # warm
        jax.block_until_ready(out)
        best = float("inf")
        for _ in range(iters):
            z = [jax.device_put(a, sh) for a in _zeros()]
            jax.block_until_ready(z)
            t0 = time.perf_counter()
            out = sharded(*staged, *z)
            jax.block_until_ready(out)
            best = min(best, time.perf_counter() - t0)
        return _unpack(out), best

    run.run_timed = run_timed
    return run


# ---------------------------------------------------------------- launch 1
def _register_consts(nc, values):
    for value in values:
        t = nc.alloc_sbuf_tensor(f"constx-f32-{value}", [128, 1], F32)
        nc.gpsimd.memset(t.ap(), value)
        nc.const_aps.aps[(F32, value)] = t.ap()
    nc.all_engine_barrier()


def build_launch1(K):
    """CN pass: padded slot matrix -> cn grid -> AllReduce -> W."""
    nc = bacc.Bacc(None, target_bir_lowering=False, num_devices=N_CORES)
    _register_consts(nc, [1e-20, K1])
    # [atom, k, field]; fields = xi yi zi rcovi xj yj zj rcovj
    p1 = nc.dram_tensor("p1", [NP_ATOMS, K, 8], F32, kind="ExternalInput")
    cnr = nc.dram_tensor("cnr", [NP_ATOMS, NREF], F32, kind="ExternalInput")
    wout = nc.dram_tensor("wout", [NP_ATOMS, NREF], F32, kind="ExternalOutput")
    cnout = nc.dram_tensor("cnout", [128, GRID_C], F32, kind="ExternalOutput")

    NCH = 17  # atom-chunks: 391 = 17*23
    CC = GRID_C // NCH  # 23 atoms/partition/chunk

    with tile.TileContext(nc) as tc:
        with (
            tc.tile_pool(name="io", bufs=3) as io,
            tc.tile_pool(name="tmp", bufs=2) as tp,
            tc.tile_pool(name="acc", bufs=1) as ac,
            tc.tile_pool(name="dram", bufs=1, space="DRAM") as dr,
        ):
            cng = ac.tile([128, GRID_C], F32)
            for ch in range(NCH):
                a0 = ch * CC
                t = io.tile([128, CC * K * 8], F32, tag="p1in")
                nc.sync.dma_start(
                    t[:],
                    p1[:, :, :]
                    .rearrange("(p c) k f -> p (c k f)", p=128)[
                        :, a0 * K * 8 : (a0 + CC) * K * 8
                    ],
                )
                v = t[:].rearrange("p (c k f) -> p (c k) f", k=K, f=8)
                S = CC * K
                dx = tp.tile([128, S], F32, tag="dx")
                dy = tp.tile([128, S], F32, tag="dy")
                d2 = tp.tile([128, S], F32, tag="d2")
                rr = tp.tile([128, S], F32, tag="rr")
                nc.vector.tensor_tensor(dx[:], v[:, :, 0], v[:, :, 4], op=AX.subtract)
                nc.vector.tensor_tensor(dy[:], v[:, :, 1], v[:, :, 5], op=AX.subtract)
                nc.vector.tensor_tensor(rr[:], v[:, :, 3], v[:, :, 7], op=AX.add)
                nc.vector.tensor_tensor(d2[:], dx[:], dx[:], op=AX.mult)
                nc.vector.tensor_tensor(dx[:], dy[:], dy[:], op=AX.mult)
                nc.vector.tensor_tensor(d2[:], d2[:], dx[:], op=AX.add)
                nc.vector.tensor_tensor(dy[:], v[:, :, 2], v[:, :, 6], op=AX.subtract)
                nc.vector.tensor_tensor(dx[:], dy[:], dy[:], op=AX.mult)
                nc.vector.tensor_tensor(d2[:], d2[:], dx[:], op=AX.add)
                # ln(d2 + 1e-20), ln(rr)
                ln_d2 = tp.tile([128, S], F32, tag="lnd2")
                ln_rr = tp.tile([128, S], F32, tag="lnrr")
                nc.scalar.activation(ln_d2[:], d2[:], ACTF.Ln, bias=1e-20)
                nc.scalar.activation(ln_rr[:], rr[:], ACTF.Ln)
                # t = exp(ln_rr - 0.5 ln_d2)  (= rr/d)
                arg = tp.tile([128, S], F32, tag="arg")
                nc.vector.tensor_scalar(arg[:], ln_d2[:], -0.5, None, op0=AX.mult)
                nc.vector.tensor_tensor(arg[:], arg[:], ln_rr[:], op=AX.add)
                tt = tp.tile([128, S], F32, tag="tt")
                nc.scalar.activation(tt[:], arg[:], ACTF.Exp)
                # g = exp(K1 - K1*K2*t); cnp = mask / (1 + g)
                g = tp.tile([128, S], F32, tag="g")
                nc.scalar.activation(g[:], tt[:], ACTF.Exp, bias=K1, scale=-K1 * K2)
                nc.vector.tensor_scalar(g[:], g[:], 1.0, None, op0=AX.add)
                rec = tp.tile([128, S], F32, tag="rec")
                nc.vector.reciprocal(rec[:], g[:])
                msk = tp.tile([128, S], F32, tag="msk")
                nc.vector.tensor_scalar(msk[:], d2[:], CN_CUTOFF2, None, op0=AX.is_lt)
                nc.vector.tensor_tensor(rec[:], rec[:], msk[:], op=AX.mult)
                # row-reduce K slots -> cn for CC atoms
                nc.vector.tensor_reduce(
                    cng[:, a0 : a0 + CC],
                    rec[:].rearrange("p (c k) -> p c k", k=K),
                    axis=mybir.AxisListType.X,
                    op=AX.add,
                )

            # AllReduce cn across cores (psum)
            cin = dr.tile([128, GRID_C], F32)
            cout = dr.tile([128, GRID_C], F32)
            nc.sync.dma_start(cin[:], cng[:])
            nc.gpsimd.collective_compute(
                "AllReduce",
                AX.add,
                replica_groups=[list(range(N_CORES))],
                ins=[cin[:].opt()],
                outs=[cout[:].opt()],
            )
            cn = ac.tile([128, GRID_C], F32)
            nc.sync.dma_start(cn[:], cout[:])
            nc.sync.dma_start(cnout[:], cn[:])

            # ---- W build (per atom) ----
            G = GRID_C
            cr = ac.tile([128, G * NREF], F32)
            nc.sync.dma_start(
                cr[:], cnr[:].rearrange("(p c) r -> p (c r)", p=128)
            )
            crv = cr[:].rearrange("p (c r) -> p c r", r=NREF)
            gw = ac.tile([128, G * NREF], F32)
            gwv = gw[:].rearrange("p (c r) -> p c r", r=NREF)
            mk = ac.tile([128, G * NREF], F32)
            mkv = mk[:].rearrange("p (c r) -> p c r", r=NREF)
            dr_ = tp.tile([128, G], F32, tag="wdr")
            for r in range(NREF):
                nc.vector.tensor_tensor(dr_[:], cn[:], crv[:, :, r], op=AX.subtract)
                nc.vector.tensor_tensor(dr_[:], dr_[:], dr_[:], op=AX.mult)
                nc.scalar.activation(gwv[:, :, r], dr_[:], ACTF.Exp, scale=-K3)
            nc.vector.tensor_scalar(mk[:], cr[:], 0.0, None, op0=AX.is_ge)
            nc.vector.tensor_tensor(gw[:], gw[:], mk[:], op=AX.mult)
            norm = tp.tile([128, G], F32, tag="wnorm")
            nc.vector.tensor_reduce(
                norm[:], gwv[:, :, :], axis=mybir.AxisListType.X, op=AX.add
            )
            # maxv = ref4 if ref4>=0 else ref3
            maxv = tp.tile([128, G], F32, tag="wmaxv")
            t1 = tp.tile([128, G], F32, tag="wt1")
            nc.vector.tensor_tensor(
                maxv[:], crv[:, :, NREF - 1], mkv[:, :, NREF - 1], op=AX.mult
            )
            nc.vector.tensor_scalar(
                t1[:], mkv[:, :, NREF - 1], -1.0, 1.0, op0=AX.mult, op1=AX.add
            )
            nc.vector.tensor_tensor(t1[:], t1[:], crv[:, :, NREF - 2], op=AX.mult)
            nc.vector.tensor_tensor(maxv[:], maxv[:], t1[:], op=AX.add)
            # usefb / denom
            usefb = tp.tile([128, G], F32, tag="wufb")
            nc.vector.tensor_scalar(usefb[:], norm[:], 1e-30, None, op0=AX.is_le)
            nofb = tp.tile([128, G], F32, tag="wnfb")
            nc.vector.tensor_scalar(
                nofb[:], usefb[:], -1.0, 1.0, op0=AX.mult, op1=AX.add
            )
            nc.vector.tensor_scalar(norm[:], norm[:], 1e-30, None, op0=AX.max)
            rn = tp.tile([128, G], F32, tag="wrn")
            nc.vector.reciprocal(rn[:], norm[:])
            nc.vector.tensor_tensor(rn[:], rn[:], nofb[:], op=AX.mult)
            wpack = ac.tile([128, G * NREF], F32)
            wv = wpack[:].rearrange("p (c r) -> p c r", r=NREF)
            fb = tp.tile([128, G], F32, tag="wfb")
            for r in range(NREF):
                nc.vector.tensor_tensor(fb[:], crv[:, :, r], maxv[:], op=AX.is_equal)
                nc.vector.tensor_tensor(fb[:], fb[:], mkv[:, :, r], op=AX.mult)
                nc.vector.tensor_tensor(fb[:], fb[:], usefb[:], op=AX.mult)
                nc.vector.tensor_tensor(
                    wv[:, :, r], gwv[:, :, r], rn[:], op=AX.mult
                )
                nc.vector.tensor_tensor(
                    wv[:, :, r], wv[:, :, r], fb[:], op=AX.add
                )
            nc.sync.dma_start(
                wout[:].rearrange("(p c) r -> p (c r)", p=128), wpack[:]
            )
    nc.finalize()
    return nc


# ---------------------------------------------------------------- launch 2
def build_launch2():
    nc = bacc.Bacc(None, target_bir_lowering=False, num_devices=N_CORES)
    # geo: xi yi zi xj yj zj r4i r4j
    geo = nc.dram_tensor("geo", [E_PAD2, 8], F32, kind="ExternalInput")
    wij = nc.dram_tensor("wij", [E_PAD2, 2 * NREF], F32, kind="ExternalInput")
    c6b = nc.dram_tensor("c6b", [E_PAD2, 25], F32, kind="ExternalInput")
    eout = nc.dram_tensor("eout", [128, 1], F32, kind="ExternalOutput")

    C = L2_C
    with tile.TileContext(nc) as tc:
        with (
            tc.tile_pool(name="io", bufs=2) as io,
            tc.tile_pool(name="tmp", bufs=2) as tp,
            tc.tile_pool(name="acc", bufs=1) as ac,
        ):
            eacc = ac.tile([128, 1], F32)
            nc.vector.memset(eacc[:], 0.0)
            for ch in range(L2_NCH):
                e0 = ch * 128 * C
                g = io.tile([128, C * 8], F32, tag="geo")
                nc.sync.dma_start(
                    g[:],
                    geo[e0 : e0 + 128 * C, :].rearrange(
                        "(p c) f -> p (c f)", p=128
                    ),
                )
                gv = g[:].rearrange("p (c f) -> p c f", f=8)
                w = io.tile([128, C * 2 * NREF], F32, tag="wij")
                nc.sync.dma_start(
                    w[:],
                    wij[e0 : e0 + 128 * C, :].rearrange(
                        "(p c) f -> p (c f)", p=128
                    ),
                )
                wvv = w[:].rearrange("p (c f) -> p c f", f=2 * NREF)
                cb = io.tile([128, C * 25], F32, tag="c6b")
                nc.sync.dma_start(
                    cb[:],
                    c6b[e0 : e0 + 128 * C, :].rearrange(
                        "(p c) f -> p (c f)", p=128
                    ),
                )
                # d2
                dx = tp.tile([128, C], F32, tag="dx")
                dy = tp.tile([128, C], F32, tag="dy")
                d2 = tp.tile([128, C], F32, tag="d2")
                nc.vector.tensor_tensor(dx[:], gv[:, :, 0], gv[:, :, 3], op=AX.subtract)
                nc.vector.tensor_tensor(dy[:], gv[:, :, 1], gv[:, :, 4], op=AX.subtract)
                nc.vector.tensor_tensor(d2[:], dx[:], dx[:], op=AX.mult)
                nc.vector.tensor_tensor(dx[:], dy[:], dy[:], op=AX.mult)
                nc.vector.tensor_tensor(d2[:], d2[:], dx[:], op=AX.add)
                nc.vector.tensor_tensor(dy[:], gv[:, :, 2], gv[:, :, 5], op=AX.subtract)
                nc.vector.tensor_tensor(dx[:], dy[:], dy[:], op=AX.mult)
                nc.vector.tensor_tensor(d2[:], d2[:], dx[:], op=AX.add)
                nc.vector.tensor_scalar(d2[:], d2[:], 1e-20, None, op0=AX.add)
                # q = r4i*r4j ; sq = sqrt(q) = exp(0.5 ln q); f = A1*sqrt(3q)+A2
                q = tp.tile([128, C], F32, tag="q")
                nc.vector.tensor_tensor(q[:], gv[:, :, 6], gv[:, :, 7], op=AX.mult)
                lnq = tp.tile([128, C], F32, tag="lnq")
                nc.scalar.activation(lnq[:], q[:], ACTF.Ln)
                sq = tp.tile([128, C], F32, tag="sq")
                nc.scalar.activation(sq[:], lnq[:], ACTF.Exp, scale=0.5)
                f = tp.tile([128, C], F32, tag="f")
                nc.vector.tensor_scalar(
                    f[:], sq[:], A1 * np.sqrt(3.0), A2, op0=AX.mult, op1=AX.add
                )
                f2 = tp.tile([128, C], F32, tag="f2")
                nc.vector.tensor_tensor(f2[:], f[:], f[:], op=AX.mult)
                f4 = tp.tile([128, C], F32, tag="f4")
                nc.vector.tensor_tensor(f4[:], f2[:], f2[:], op=AX.mult)
                f6 = tp.tile([128, C], F32, tag="f6")
                nc.vector.tensor_tensor(f6[:], f4[:], f2[:], op=AX.mult)
                nc.vector.tensor_tensor(f4[:], f4[:], f4[:], op=AX.mult)  # f8
                d4 = tp.tile([128, C], F32, tag="d4")
                nc.vector.tensor_tensor(d4[:], d2[:], d2[:], op=AX.mult)
                d6 = tp.tile([128, C], F32, tag="d6")
                nc.vector.tensor_tensor(d6[:], d4[:], d2[:], op=AX.mult)
                nc.vector.tensor_tensor(d4[:], d4[:], d4[:], op=AX.mult)  # d8
                nc.vector.tensor_tensor(d6[:], d6[:], f6[:], op=AX.add)
                nc.vector.tensor_tensor(d4[:], d4[:], f4[:], op=AX.add)
                r6 = tp.tile([128, C], F32, tag="r6")
                nc.vector.reciprocal(r6[:], d6[:])
                r8 = tp.tile([128, C], F32, tag="r8")
                nc.vector.reciprocal(r8[:], d4[:])
                # u = (S6*r6 + 3*S8*q*r8) * (d2<2500)   [sign applied at end]
                nc.vector.tensor_tensor(r8[:], r8[:], q[:], op=AX.mult)
                nc.vector.tensor_scalar(r8[:], r8[:], 3.0 * S8, None, op0=AX.mult)
                nc.vector.tensor_scalar(r6[:], r6[:], S6, None, op0=AX.mult)
                nc.vector.tensor_tensor(r6[:], r6[:], r8[:], op=AX.add)
                m50 = tp.tile([128, C], F32, tag="m50")
                nc.vector.tensor_scalar(
                    m50[:], d2[:], DISP_CUTOFF2, None, op0=AX.is_lt
                )
                nc.vector.tensor_tensor(r6[:], r6[:], m50[:], op=AX.mult)
                # einsum: c6 = sum_ab Wi_a Wj_b B_ab
                op = tp.tile([128, C * 25], F32, tag="op")
                opv = op[:].rearrange("p (c a b) -> p c a b", a=NREF, b=NREF)
                for a in range(NREF):
                    for b in range(NREF):
                        nc.vector.tensor_tensor(
                            opv[:, :, a, b],
                            wvv[:, :, a],
                            wvv[:, :, NREF + b],
                            op=AX.mult,
                        )
                nc.vector.tensor_tensor(op[:], op[:], cb[:], op=AX.mult)
                c6 = tp.tile([128, C], F32, tag="c6")
                nc.vector.tensor_reduce(
                    c6[:],
                    op[:].rearrange("p (c e) -> p c e", e=25),
                    axis=mybir.AxisListType.X,
                    op=AX.add,
                )
                nc.vector.tensor_tensor(c6[:], c6[:], r6[:], op=AX.mult)
                er = tp.tile([128, 1], F32, tag="er")
                nc.vector.tensor_reduce(
                    er[:], c6[:], axis=mybir.AxisListType.X, op=AX.add
                )
                nc.vector.tensor_tensor(eacc[:], eacc[:], er[:], op=AX.add)
            nc.vector.tensor_scalar(eacc[:], eacc[:], -0.5, None, op0=AX.mult)
            nc.sync.dma_start(eout[:], eacc[:])
    nc.finalize()
    return nc


# ---------------------------------------------------------------- host side
def _prep(positions, numbers, edges_i, edges_j, rcov, r4r2, c6_table, cn_ref):
    """Host-side sharding + index marshalling. Returns (K, l1_maps, meta)."""
    pos = np.zeros((NP_ATOMS, 3), np.float32)
    pos[:N_ATOMS] = positions
    # pad atoms far away so any accidental reference is masked out
    pos[N_ATOMS:] = 1.0e4
    num = np.zeros(NP_ATOMS, np.int32)
    num[:N_ATOMS] = numbers
    rcov_a = rcov[num].astype(np.float32)
    r4r2_a = r4r2[num].astype(np.float32)
    cnr_a = cn_ref[num].astype(np.float32)  # [NP, 5]

    cores = []
    Kmax = 1
    for c in range(N_CORES):
        ei = edges_i[c * E_CORE : (c + 1) * E_CORE].astype(np.int64)
        ej = edges_j[c * E_CORE : (c + 1) * E_CORE].astype(np.int64)
        order = np.argsort(ei, kind="stable")
        ei, ej = ei[order], ej[order]
        counts = np.bincount(ei, minlength=NP_ATOMS)
        Kmax = max(Kmax, int(counts.max()))
        cores.append((ei, ej, counts))
    K = int(Kmax)

    l1_maps = []
    metas = []
    for c in range(N_CORES):
        ei, ej, counts = cores[c]
        starts = np.zeros(NP_ATOMS, np.int64)
        starts[1:] = np.cumsum(counts)[:-1]
        kpos = np.arange(E_CORE, dtype=np.int64) - starts[ei]
        slot = ei * K + kpos
        p1 = np.zeros((NP_ATOMS * K, 8), np.float32)
        # pad slots: coincident points at huge coords -> d2=1e-20? No:
        # default zeros give d2=1e-20 -> mask (d2<625) passes -> rr=0
        # -> t=exp(ln 0 - ...)=0 -> sigma tiny but nonzero! Set pad xj=1e3.
        p1[:, 4] = 1.0e3
        p1[:, 3] = 0.5
        p1[:, 7] = 0.5
        p1[slot, 0:3] = pos[ei]
        p1[slot, 3] = rcov_a[ei]
        p1[slot, 4:7] = pos[ej]
        p1[slot, 7] = rcov_a[ej]
        l1_maps.append(
            dict(p1=p1.reshape(NP_ATOMS, K, 8), cnr=cnr_a)
        )
        metas.append((ei, ej))
    return K, l1_maps, metas


def kernel(positions, numbers, edges_i, edges_j, rcov, r4r2, c6_table,
           cn_ref, _times=None):
    K, l1_maps, metas = _prep(
        positions, numbers, edges_i, edges_j, rcov, r4r2, c6_table, cn_ref
    )

    if ("l1", K) not in _cache:
        _cache[("l1", K)] = _runner(build_launch1(K), ["wout", "cnout"])
    run1 = _cache[("l1", K)]
    if _times is not None:
        res1, t1 = run1.run_timed(l1_maps)
        _times.append(t1)
    else:
        res1 = run1(l1_maps)
    W = res1[0]["wout"]  # [NP_ATOMS, 5] (identical on all cores)

    num = np.zeros(NP_ATOMS, np.int32)
    num[:N_ATOMS] = numbers
    pos = np.zeros((NP_ATOMS, 3), np.float32)
    pos[:N_ATOMS] = positions
    r4r2_a = r4r2[num].astype(np.float32)
    c6f = np.ascontiguousarray(c6_table.reshape(95 * 95, 25).astype(np.float32))

    l2_maps = []
    for c in range(N_CORES):
        ei, ej = metas[c]
        geo = np.zeros((E_PAD2, 8), np.float32)
        geo[:, 3] = 1.0e3  # pad: far apart -> masked
        geo[:, 6:8] = 1.0  # pad: ln(1)=0 safe
        geo[:E_CORE, 0:3] = pos[ei]
        geo[:E_CORE, 3:6] = pos[ej]
        geo[:E_CORE, 6] = r4r2_a[ei]
        geo[:E_CORE, 7] = r4r2_a[ej]
        wij = np.zeros((E_PAD2, 10), np.float32)
        wij[:E_CORE, 0:5] = W[ei]
        wij[:E_CORE, 5:10] = W[ej]
        c6b = np.zeros((E_PAD2, 25), np.float32)
        pair = num[ei].astype(np.int64) * 95 + num[ej]
        c6b[:E_CORE] = c6f[pair]
        l2_maps.append(dict(geo=geo, wij=wij, c6b=c6b))

    if "l2" not in _cache:
        _cache["l2"] = _runner(build_launch2(), ["eout"])
    run2 = _cache["l2"]
    if _times is not None:
        res2, t2 = run2.run_timed(l2_maps)
        _times.append(t2)
    else:
        res2 = run2(l2_maps)
    total = sum(float(res2[c]["eout"].sum()) for c in range(N_CORES))
    return np.float32(total)


# revision 12
# speedup vs baseline: 1.3150x; 1.3150x over previous
"""DFT-D3 dispersion energy kernel for 8 Trainium2 NeuronCores.

Strategy (per sharding hint): shard the 1.6M-edge list across 8 cores
(200k edges each), replicate atoms/tables. Two device launches:

  Launch 1 (CN): edges sorted by i-atom on host into a padded
    [50048, K] slot matrix per core; device computes the D3
    coordination-number counting function per slot, dense-reduces rows
    to per-atom CN partials, AllReduce-psums CN across the 8 cores, and
    computes the per-atom Gaussian C6-interpolation weights W[50048,5].

  Host: gathers W rows to edge endpoints (index marshalling only).

  Launch 2 (energy): plain per-edge arrays; device computes BJ-damped
    pair energies e = c6_ij * u(d) with c6_ij = Wi^T B Wj (B = gathered
    5x5 C6 block), reduces to per-core partials; host sums partials.

All transcendentals use the {Ln, Exp} activation table set only
(sigmoid via exp, sqrt via exp(0.5 ln)) so there is a single ACT table
load in the whole kernel.
"""

import sys

sys.path.insert(0, "/opt/trn_rl_repo")

import numpy as np

import concourse.bacc as bacc
import concourse.bass as bass
import concourse.mybir as mybir
import concourse.tile as tile
from concourse import bass_utils

F32 = mybir.dt.float32
AX = mybir.AluOpType
ACTF = mybir.ActivationFunctionType

# Our only transcendentals are Ln and Exp. Steer the ACT table-load pass
# to the combined natural_log_exp set so the kernel needs exactly one
# table load instead of thrashing between the ln-only and exp-only sets
# (~2.7us per reload).
_orig_get_tables = bacc.get_activation_tables


def _ln_exp_tables(module_arch):
    tables = dict(_orig_get_tables(module_arch))
    out = {}
    for name, funcs in tables.items():
        if name == "natural_log_exp_and_others":
            out[name] = funcs
        else:
            out[name] = funcs - {ACTF.Ln, ACTF.Exp}
    return out


bacc.get_activation_tables = _ln_exp_tables

# D3 constants
K1 = 16.0
K2 = 4.0 / 3.0
K3 = 4.0
A1, A2, S6, S8 = 0.4, 5.0, 1.0, 0.78
CN_CUTOFF2 = 25.0 * 25.0
DISP_CUTOFF2 = 50.0 * 50.0

N_ATOMS = 50000
NP_ATOMS = 50048  # = 128 * 391
GRID_C = 391
N_EDGES = 1_600_000
N_CORES = 8
E_CORE = N_EDGES // N_CORES  # 200000
NREF = 5

# launch-2 chunking: slots per partition per chunk
L2_C = 200
L2_NCH = 8  # 128*200*8 = 204800 >= 200000
E_PAD2 = 128 * L2_C * L2_NCH

_cache = {}


def _runner(nc, out_names):
    """Compile once, return a callable(in_maps) -> list of out dicts."""
    import jax
    from jax.sharding import Mesh, PartitionSpec
    from jax.experimental.shard_map import shard_map
    from concourse import bass2jax

    bass2jax.install_neuronx_cc_hook()

    partition_name = (
        nc.partition_id_tensor.name if nc.partition_id_tensor else None
    )
    in_names = []
    out_avals = []
    zero_outs = []
    onames = []
    for alloc in nc.m.functions[0].allocations:
        if not isinstance(alloc, mybir.MemoryLocationSet):
            continue
        name = alloc.memorylocations[0].name
        if alloc.kind == "ExternalInput":
            if name != partition_name:
                in_names.append(name)
        elif alloc.kind == "ExternalOutput":
            shape = list(alloc.tensor_shape)
            dt = mybir.dt.np(alloc.dtype)
            onames.append(name)
            out_avals.append(jax.core.ShapedArray(shape, dt))
            zero_outs.append(np.zeros(shape, dt))
    n_params = len(in_names)
    all_in = list(in_names) + list(onames)
    if partition_name is not None:
        all_in.append(partition_name)

    from concourse.bass2jax import _bass_exec_p, partition_id_tensor

    def _body(*args):
        operands = list(args)
        if partition_name is not None:
            operands.append(partition_id_tensor())
        outs = _bass_exec_p.bind(
            *operands,
            out_avals=tuple(out_avals),
            in_names=tuple(all_in),
            out_names=tuple(onames),
            lowering_input_output_aliases=(),
            sim_require_finite=True,
            sim_require_nnan=True,
            nc=nc,
        )
        return tuple(outs)

    devices = jax.devices()[:N_CORES]
    mesh = Mesh(np.asarray(devices), ("core",))
    donate = tuple(range(n_params, n_params + len(onames)))
    sharded = jax.jit(
        shard_map(
            _body,
            mesh=mesh,
            in_specs=(PartitionSpec("core"),) * (n_params + len(onames)),
            out_specs=(PartitionSpec("core"),) * len(onames),
            check_rep=False,
        ),
        donate_argnums=donate,
        keep_unused=True,
    )

    def _concat(in_maps):
        per_core = [[np.asarray(m[n]) for n in in_names] for m in in_maps]
        return [
            np.concatenate([per_core[c][i] for c in range(N_CORES)], axis=0)
            for i in range(n_params)
        ]

    def _zeros():
        return [
            np.zeros((N_CORES * z.shape[0], *z.shape[1:]), z.dtype)
            for z in zero_outs
        ]

    def _unpack(out_arrs):
        return [
            {
                n: np.asarray(out_arrs[i]).reshape(
                    N_CORES, *out_avals[i].shape
                )[c]
                for i, n in enumerate(onames)
            }
            for c in range(N_CORES)
        ]

    def run(in_maps):
        return _unpack(sharded(*_concat(in_maps), *_zeros()))

    def run_timed(in_maps, iters=3):
        """Pre-stage inputs on device, time execute-only. Returns
        (results, best_seconds)."""
        import time
        from jax.sharding import NamedSharding

        sh = NamedSharding(mesh, PartitionSpec("core"))
        staged = [jax.device_put(a, sh) for a in _concat(in_maps)]
        out = sharded(*staged, *_zeros())  # warm
        jax.block_until_ready(out)
        best = float("inf")
        for _ in range(iters):
            z = [jax.device_put(a, sh) for a in _zeros()]
            jax.block_until_ready(z)
            t0 = time.perf_counter()
            out = sharded(*staged, *z)
            jax.block_until_ready(out)
            best = min(best, time.perf_counter() - t0)
        return _unpack(out), best

    run.run_timed = run_timed
    return run


# ---------------------------------------------------------------- launch 1
def _register_consts(nc, values):
    for value in values:
        t = nc.alloc_sbuf_tensor(f"constx-f32-{value}", [128, 1], F32)
        nc.gpsimd.memset(t.ap(), value)
        nc.const_aps.aps[(F32, value)] = t.ap()
    nc.all_engine_barrier()


def build_launch1(K):
    """CN pass: padded slot matrix -> cn grid -> AllReduce -> W.

    k-major layout: pjt[k, atom, 4] (j-side per slot), slf[atom, 4]
    (i-side, constant per atom, read via broadcast APs). Compute runs
    full-width [128, Kc*391] per chunk to amortize DVE instruction
    overhead.
    """
    nc = bacc.Bacc(None, target_bir_lowering=False, num_devices=N_CORES)
    _register_consts(nc, [1e-20, K1])
    pjt = nc.dram_tensor("pjt", [K, NP_ATOMS, 4], F32, kind="ExternalInput")
    slf = nc.dram_tensor("slf", [NP_ATOMS, 4], F32, kind="ExternalInput")
    cnr = nc.dram_tensor("cnr", [NP_ATOMS, NREF], F32, kind="ExternalInput")
    wout = nc.dram_tensor("wout", [NP_ATOMS, NREF], F32, kind="ExternalOutput")
    cnout = nc.dram_tensor("cnout", [128, GRID_C], F32, kind="ExternalOutput")

    KC = 4  # k-slots per chunk
    G = GRID_C

    with tile.TileContext(nc) as tc:
        with (
            tc.tile_pool(name="io", bufs=2) as io,
            tc.tile_pool(name="tmp", bufs=1) as tp,
            tc.tile_pool(name="acc", bufs=1) as ac,
            tc.tile_pool(name="dram", bufs=1, space="DRAM") as dr,
        ):
            sl = ac.tile([128, G * 4], F32)
            nc.sync.dma_start(
                sl[:], slf[:].rearrange("(p c) f -> p (c f)", p=128)
            )
            slv = sl[:].rearrange("p (c f) -> p c f", f=4)

            def selfb(f, kc):
                # [128, G] field -> [128, kc, G] broadcast over k
                return (
                    slv[:, :, f]
                    .to_broadcast([128, G, kc])
                    .rearrange("p c k -> p k c")
                )

            cng = ac.tile([128, GRID_C], F32)
            nc.vector.memset(cng[:], 0.0)
            k0 = 0
            while k0 < K:
                kc = min(KC, K - k0)
                t = io.tile([128, KC * G * 4], F32, tag="pjin")
                for ki in range(kc):
                    nc.sync.dma_start(
                        t[:].rearrange("p (k m) -> p k m", k=KC)[:, ki, :],
                        pjt[k0 + ki].rearrange("(p c) f -> p (c f)", p=128),
                    )
                v = t[:].rearrange("p (k c f) -> p k c f", k=KC, f=4)[:, :kc]
                S = kc * G
                dx = tp.tile([128, KC * G], F32, tag="dx")
                dy = tp.tile([128, KC * G], F32, tag="dy")
                d2 = tp.tile([128, KC * G], F32, tag="d2")
                rr = tp.tile([128, KC * G], F32, tag="rr")
                dxv = dx[:, :S].rearrange("p (k c) -> p k c", k=kc)
                dyv = dy[:, :S].rearrange("p (k c) -> p k c", k=kc)
                d2v = d2[:, :S].rearrange("p (k c) -> p k c", k=kc)
                rrv = rr[:, :S].rearrange("p (k c) -> p k c", k=kc)
                nc.vector.tensor_tensor(dxv, v[:, :, :, 0], selfb(0, kc), op=AX.subtract)
                nc.vector.tensor_tensor(dyv, v[:, :, :, 1], selfb(1, kc), op=AX.subtract)
                nc.vector.tensor_tensor(rrv, v[:, :, :, 3], selfb(3, kc), op=AX.add)
                nc.vector.tensor_tensor(d2[:, :S], dx[:, :S], dx[:, :S], op=AX.mult)
                nc.vector.tensor_tensor(dx[:, :S], dy[:, :S], dy[:, :S], op=AX.mult)
                nc.vector.tensor_tensor(d2[:, :S], d2[:, :S], dx[:, :S], op=AX.add)
                nc.vector.tensor_tensor(dyv, v[:, :, :, 2], selfb(2, kc), op=AX.subtract)
                nc.vector.tensor_tensor(dx[:, :S], dy[:, :S], dy[:, :S], op=AX.mult)
                nc.vector.tensor_tensor(d2[:, :S], d2[:, :S], dx[:, :S], op=AX.add)
                ln_d2 = tp.tile([128, KC * G], F32, tag="lnd2")
                ln_rr = tp.tile([128, KC * G], F32, tag="lnrr")
                nc.scalar.activation(ln_d2[:, :S], d2[:, :S], ACTF.Ln, bias=1e-20)
                nc.scalar.activation(ln_rr[:, :S], rr[:, :S], ACTF.Ln)
                arg = tp.tile([128, KC * G], F32, tag="arg")
                nc.vector.tensor_scalar(arg[:, :S], ln_d2[:, :S], -0.5, None, op0=AX.mult)
                nc.vector.tensor_tensor(arg[:, :S], arg[:, :S], ln_rr[:, :S], op=AX.add)
                tt = tp.tile([128, KC * G], F32, tag="tt")
                nc.scalar.activation(tt[:, :S], arg[:, :S], ACTF.Exp)
                g = tp.tile([128, KC * G], F32, tag="g")
                nc.scalar.activation(g[:, :S], tt[:, :S], ACTF.Exp, bias=K1, scale=-K1 * K2)
                nc.vector.tensor_scalar(g[:, :S], g[:, :S], 1.0, None, op0=AX.add)
                rec = tp.tile([128, KC * G], F32, tag="rec")
                nc.vector.reciprocal(rec[:, :S], g[:, :S])
                msk = tp.tile([128, KC * G], F32, tag="msk")
                nc.vector.tensor_scalar(msk[:, :S], d2[:, :S], CN_CUTOFF2, None, op0=AX.is_lt)
                nc.vector.tensor_tensor(rec[:, :S], rec[:, :S], msk[:, :S], op=AX.mult)
                # reduce over k (strided innermost) and accumulate
                part = tp.tile([128, G], F32, tag="part")
                nc.vector.tensor_reduce(
                    part[:],
                    rec[:, :S]
                    .rearrange("p (k c) -> p k c", k=kc)
                    .rearrange("p k c -> p c k"),
                    axis=mybir.AxisListType.X,
                    op=AX.add,
                )
                nc.vector.tensor_tensor(cng[:], cng[:], part[:], op=AX.add)
                k0 += kc

            # AllReduce cn across cores (psum)
            cin = dr.tile([128, GRID_C], F32)
            cout = dr.tile([128, GRID_C], F32)
            nc.sync.dma_start(cin[:], cng[:])
            nc.gpsimd.collective_compute(
                "AllReduce",
                AX.add,
                replica_groups=[list(range(N_CORES))],
                ins=[cin[:].opt()],
                outs=[cout[:].opt()],
            )
            cn = ac.tile([128, GRID_C], F32)
            nc.sync.dma_start(cn[:], cout[:])
            nc.sync.dma_start(cnout[:], cn[:])

            # ---- W build (per atom) ----
            G = GRID_C
            cr = ac.tile([128, G * NREF], F32)
            nc.sync.dma_start(
                cr[:], cnr[:].rearrange("(p c) r -> p (c r)", p=128)
            )
            crv = cr[:].rearrange("p (c r) -> p c r", r=NREF)
            gw = ac.tile([128, G * NREF], F32)
            gwv = gw[:].rearrange("p (c r) -> p c r", r=NREF)
            mk = ac.tile([128, G * NREF], F32)
            mkv = mk[:].rearrange("p (c r) -> p c r", r=NREF)
            dr_ = tp.tile([128, G], F32, tag="wdr")
            for r in range(NREF):
                nc.vector.tensor_tensor(dr_[:], cn[:], crv[:, :, r], op=AX.subtract)
                nc.vector.tensor_tensor(dr_[:], dr_[:], dr_[:], op=AX.mult)
                nc.scalar.activation(gwv[:, :, r], dr_[:], ACTF.Exp, scale=-K3)
            nc.vector.tensor_scalar(mk[:], cr[:], 0.0, None, op0=AX.is_ge)
            nc.vector.tensor_tensor(gw[:], gw[:], mk[:], op=AX.mult)
            norm = tp.tile([128, G], F32, tag="wnorm")
            nc.vector.tensor_reduce(
                norm[:], gwv[:, :, :], axis=mybir.AxisListType.X, op=AX.add
            )
            # maxv = ref4 if ref4>=0 else ref3
            maxv = tp.tile([128, G], F32, tag="wmaxv")
            t1 = tp.tile([128, G], F32, tag="wt1")
            nc.vector.tensor_tensor(
                maxv[:], crv[:, :, NREF - 1], mkv[:, :, NREF - 1], op=AX.mult
            )
            nc.vector.tensor_scalar(
                t1[:], mkv[:, :, NREF - 1], -1.0, 1.0, op0=AX.mult, op1=AX.add
            )
            nc.vector.tensor_tensor(t1[:], t1[:], crv[:, :, NREF - 2], op=AX.mult)
            nc.vector.tensor_tensor(maxv[:], maxv[:], t1[:], op=AX.add)
            # usefb / denom
            usefb = tp.tile([128, G], F32, tag="wufb")
            nc.vector.tensor_scalar(usefb[:], norm[:], 1e-30, None, op0=AX.is_le)
            nofb = tp.tile([128, G], F32, tag="wnfb")
            nc.vector.tensor_scalar(
                nofb[:], usefb[:], -1.0, 1.0, op0=AX.mult, op1=AX.add
            )
            nc.vector.tensor_scalar(norm[:], norm[:], 1e-30, None, op0=AX.max)
            rn = tp.tile([128, G], F32, tag="wrn")
            nc.vector.reciprocal(rn[:], norm[:])
            nc.vector.tensor_tensor(rn[:], rn[:], nofb[:], op=AX.mult)
            wpack = ac.tile([128, G * NREF], F32)
            wv = wpack[:].rearrange("p (c r) -> p c r", r=NREF)
            fb = tp.tile([128, G], F32, tag="wfb")
            for r in range(NREF):
                nc.vector.tensor_tensor(fb[:], crv[:, :, r], maxv[:], op=AX.is_equal)
                nc.vector.tensor_tensor(fb[:], fb[:], mkv[:, :, r], op=AX.mult)
                nc.vector.tensor_tensor(fb[:], fb[:], usefb[:], op=AX.mult)
                nc.vector.tensor_tensor(
                    wv[:, :, r], gwv[:, :, r], rn[:], op=AX.mult
                )
                nc.vector.tensor_tensor(
                    wv[:, :, r], wv[:, :, r], fb[:], op=AX.add
                )
            nc.sync.dma_start(
                wout[:].rearrange("(p c) r -> p (c r)", p=128), wpack[:]
            )
    nc.finalize()
    return nc


# ---------------------------------------------------------------- launch 2
def build_launch2():
    nc = bacc.Bacc(None, target_bir_lowering=False, num_devices=N_CORES)
    # geo: xi yi zi xj yj zj r4i r4j
    geo = nc.dram_tensor("geo", [E_PAD2, 8], F32, kind="ExternalInput")
    wij = nc.dram_tensor("wij", [E_PAD2, 2 * NREF], F32, kind="ExternalInput")
    c6b = nc.dram_tensor("c6b", [E_PAD2, 25], F32, kind="ExternalInput")
    eout = nc.dram_tensor("eout", [128, 1], F32, kind="ExternalOutput")

    C = L2_C
    with tile.TileContext(nc) as tc:
        with (
            tc.tile_pool(name="io", bufs=2) as io,
            tc.tile_pool(name="tmp", bufs=2) as tp,
            tc.tile_pool(name="acc", bufs=1) as ac,
        ):
            eacc = ac.tile([128, 1], F32)
            nc.vector.memset(eacc[:], 0.0)
            for ch in range(L2_NCH):
                e0 = ch * 128 * C
                g = io.tile([128, C * 8], F32, tag="geo")
                nc.sync.dma_start(
                    g[:],
                    geo[e0 : e0 + 128 * C, :].rearrange(
                        "(p c) f -> p (c f)", p=128
                    ),
                )
                gv = g[:].rearrange("p (c f) -> p c f", f=8)
                w = io.tile([128, C * 2 * NREF], F32, tag="wij")
                nc.sync.dma_start(
                    w[:],
                    wij[e0 : e0 + 128 * C, :].rearrange(
                        "(p c) f -> p (c f)", p=128
                    ),
                )
                wvv = w[:].rearrange("p (c f) -> p c f", f=2 * NREF)
                cb = io.tile([128, C * 25], F32, tag="c6b")
                nc.sync.dma_start(
                    cb[:],
                    c6b[e0 : e0 + 128 * C, :].rearrange(
                        "(p c) f -> p (c f)", p=128
                    ),
                )
                # d2
                dx = tp.tile([128, C], F32, tag="dx")
                dy = tp.tile([128, C], F32, tag="dy")
                d2 = tp.tile([128, C], F32, tag="d2")
                nc.vector.tensor_tensor(dx[:], gv[:, :, 0], gv[:, :, 3], op=AX.subtract)
                nc.vector.tensor_tensor(dy[:], gv[:, :, 1], gv[:, :, 4], op=AX.subtract)
                nc.vector.tensor_tensor(d2[:], dx[:], dx[:], op=AX.mult)
                nc.vector.tensor_tensor(dx[:], dy[:], dy[:], op=AX.mult)
                nc.vector.tensor_tensor(d2[:], d2[:], dx[:], op=AX.add)
                nc.vector.tensor_tensor(dy[:], gv[:, :, 2], gv[:, :, 5], op=AX.subtract)
                nc.vector.tensor_tensor(dx[:], dy[:], dy[:], op=AX.mult)
                nc.vector.tensor_tensor(d2[:], d2[:], dx[:], op=AX.add)
                nc.vector.tensor_scalar(d2[:], d2[:], 1e-20, None, op0=AX.add)
                # q = r4i*r4j ; sq = sqrt(q) = exp(0.5 ln q); f = A1*sqrt(3q)+A2
                q = tp.tile([128, C], F32, tag="q")
                nc.vector.tensor_tensor(q[:], gv[:, :, 6], gv[:, :, 7], op=AX.mult)
                lnq = tp.tile([128, C], F32, tag="lnq")
                nc.scalar.activation(lnq[:], q[:], ACTF.Ln)
                sq = tp.tile([128, C], F32, tag="sq")
                nc.scalar.activation(sq[:], lnq[:], ACTF.Exp, scale=0.5)
                f = tp.tile([128, C], F32, tag="f")
                nc.vector.tensor_scalar(
                    f[:], sq[:], A1 * np.sqrt(3.0), A2, op0=AX.mult, op1=AX.add
                )
                f2 = tp.tile([128, C], F32, tag="f2")
                nc.vector.tensor_tensor(f2[:], f[:], f[:], op=AX.mult)
                f4 = tp.tile([128, C], F32, tag="f4")
                nc.vector.tensor_tensor(f4[:], f2[:], f2[:], op=AX.mult)
                f6 = tp.tile([128, C], F32, tag="f6")
                nc.vector.tensor_tensor(f6[:], f4[:], f2[:], op=AX.mult)
                nc.vector.tensor_tensor(f4[:], f4[:], f4[:], op=AX.mult)  # f8
                d4 = tp.tile([128, C], F32, tag="d4")
                nc.vector.tensor_tensor(d4[:], d2[:], d2[:], op=AX.mult)
                d6 = tp.tile([128, C], F32, tag="d6")
                nc.vector.tensor_tensor(d6[:], d4[:], d2[:], op=AX.mult)
                nc.vector.tensor_tensor(d4[:], d4[:], d4[:], op=AX.mult)  # d8
                nc.vector.tensor_tensor(d6[:], d6[:], f6[:], op=AX.add)
                nc.vector.tensor_tensor(d4[:], d4[:], f4[:], op=AX.add)
                r6 = tp.tile([128, C], F32, tag="r6")
                nc.vector.reciprocal(r6[:], d6[:])
                r8 = tp.tile([128, C], F32, tag="r8")
                nc.vector.reciprocal(r8[:], d4[:])
                # u = (S6*r6 + 3*S8*q*r8) * (d2<2500)   [sign applied at end]
                nc.vector.tensor_tensor(r8[:], r8[:], q[:], op=AX.mult)
                nc.vector.tensor_scalar(r8[:], r8[:], 3.0 * S8, None, op0=AX.mult)
                nc.vector.tensor_scalar(r6[:], r6[:], S6, None, op0=AX.mult)
                nc.vector.tensor_tensor(r6[:], r6[:], r8[:], op=AX.add)
                m50 = tp.tile([128, C], F32, tag="m50")
                nc.vector.tensor_scalar(
                    m50[:], d2[:], DISP_CUTOFF2, None, op0=AX.is_lt
                )
                nc.vector.tensor_tensor(r6[:], r6[:], m50[:], op=AX.mult)
                # einsum: c6 = sum_ab Wi_a Wj_b B_ab
                op = tp.tile([128, C * 25], F32, tag="op")
                opv = op[:].rearrange("p (c a b) -> p c a b", a=NREF, b=NREF)
                wiB = wvv[:, :, 0:NREF].to_broadcast([128, C, NREF, NREF])
                wjB = (
                    wvv[:, :, NREF : 2 * NREF]
                    .to_broadcast([128, C, NREF, NREF])
                    .rearrange("p c b a -> p c a b")
                )
                nc.vector.tensor_tensor(opv, wiB, wjB, op=AX.mult)
                nc.vector.tensor_tensor(op[:], op[:], cb[:], op=AX.mult)
                c6 = tp.tile([128, C], F32, tag="c6")
                nc.vector.tensor_reduce(
                    c6[:],
                    op[:].rearrange("p (c e) -> p c e", e=25),
                    axis=mybir.AxisListType.X,
                    op=AX.add,
                )
                nc.vector.tensor_tensor(c6[:], c6[:], r6[:], op=AX.mult)
                er = tp.tile([128, 1], F32, tag="er")
                nc.vector.tensor_reduce(
                    er[:], c6[:], axis=mybir.AxisListType.X, op=AX.add
                )
                nc.vector.tensor_tensor(eacc[:], eacc[:], er[:], op=AX.add)
            nc.vector.tensor_scalar(eacc[:], eacc[:], -0.5, None, op0=AX.mult)
            nc.sync.dma_start(eout[:], eacc[:])
    nc.finalize()
    return nc


# ---------------------------------------------------------------- host side
def _prep(positions, numbers, edges_i, edges_j, rcov, r4r2, c6_table, cn_ref):
    """Host-side sharding + index marshalling. Returns (K, l1_maps, meta)."""
    pos = np.zeros((NP_ATOMS, 3), np.float32)
    pos[:N_ATOMS] = positions
    # pad atoms far away so any accidental reference is masked out
    pos[N_ATOMS:] = 1.0e4
    num = np.zeros(NP_ATOMS, np.int32)
    num[:N_ATOMS] = numbers
    rcov_a = rcov[num].astype(np.float32)
    r4r2_a = r4r2[num].astype(np.float32)
    cnr_a = cn_ref[num].astype(np.float32)  # [NP, 5]

    cores = []
    Kmax = 1
    for c in range(N_CORES):
        ei = edges_i[c * E_CORE : (c + 1) * E_CORE].astype(np.int64)
        ej = edges_j[c * E_CORE : (c + 1) * E_CORE].astype(np.int64)
        order = np.argsort(ei, kind="stable")
        ei, ej = ei[order], ej[order]
        counts = np.bincount(ei, minlength=NP_ATOMS)
        Kmax = max(Kmax, int(counts.max()))
        cores.append((ei, ej, counts))
    K = int(Kmax)

    l1_maps = []
    metas = []
    for c in range(N_CORES):
        ei, ej, counts = cores[c]
        starts = np.zeros(NP_ATOMS, np.int64)
        starts[1:] = np.cumsum(counts)[:-1]
        kpos = np.arange(E_CORE, dtype=np.int64) - starts[ei]
        # k-major j-side slots [K, NP, 4]; pad xj=1e3 (masked), rcov=0.5
        pjt = np.zeros((K, NP_ATOMS, 4), np.float32)
        pjt[:, :, 0] = 1.0e3
        pjt[:, :, 3] = 0.5
        pjt[kpos, ei, 0:3] = pos[ej]
        pjt[kpos, ei, 3] = rcov_a[ej]
        slfa = np.zeros((NP_ATOMS, 4), np.float32)
        slfa[:, 0:3] = pos
        slfa[:, 3] = rcov_a
        l1_maps.append(dict(pjt=pjt, slf=slfa, cnr=cnr_a))
        metas.append((ei, ej))
    return K, l1_maps, metas


def kernel(positions, numbers, edges_i, edges_j, rcov, r4r2, c6_table,
           cn_ref, _times=None):
    K, l1_maps, metas = _prep(
        positions, numbers, edges_i, edges_j, rcov, r4r2, c6_table, cn_ref
    )

    if ("l1", K) not in _cache:
        _cache[("l1", K)] = _runner(build_launch1(K), ["wout", "cnout"])
    run1 = _cache[("l1", K)]
    if _times is not None:
        res1, t1 = run1.run_timed(l1_maps)
        _times.append(t1)
    else:
        res1 = run1(l1_maps)
    W = res1[0]["wout"]  # [NP_ATOMS, 5] (identical on all cores)

    num = np.zeros(NP_ATOMS, np.int32)
    num[:N_ATOMS] = numbers
    pos = np.zeros((NP_ATOMS, 3), np.float32)
    pos[:N_ATOMS] = positions
    r4r2_a = r4r2[num].astype(np.float32)
    c6f = np.ascontiguousarray(c6_table.reshape(95 * 95, 25).astype(np.float32))

    l2_maps = []
    for c in range(N_CORES):
        ei, ej = metas[c]
        geo = np.zeros((E_PAD2, 8), np.float32)
        geo[:, 3] = 1.0e3  # pad: far apart -> masked
        geo[:, 6:8] = 1.0  # pad: ln(1)=0 safe
        geo[:E_CORE, 0:3] = pos[ei]
        geo[:E_CORE, 3:6] = pos[ej]
        geo[:E_CORE, 6] = r4r2_a[ei]
        geo[:E_CORE, 7] = r4r2_a[ej]
        wij = np.zeros((E_PAD2, 10), np.float32)
        wij[:E_CORE, 0:5] = W[ei]
        wij[:E_CORE, 5:10] = W[ej]
        c6b = np.zeros((E_PAD2, 25), np.float32)
        pair = num[ei].astype(np.int64) * 95 + num[ej]
        c6b[:E_CORE] = c6f[pair]
        l2_maps.append(dict(geo=geo, wij=wij, c6b=c6b))

    if "l2" not in _cache:
        _cache["l2"] = _runner(build_launch2(), ["eout"])
    run2 = _cache["l2"]
    if _times is not None:
        res2, t2 = run2.run_timed(l2_maps)
        _times.append(t2)
    else:
        res2 = run2(l2_maps)
    total = sum(float(res2[c]["eout"].sum()) for c in range(N_CORES))
    return np.float32(total)


# revision 19
# speedup vs baseline: 1.3333x; 1.0139x over previous
"""DFT-D3 dispersion energy kernel for 8 Trainium2 NeuronCores.

Strategy (per sharding hint): shard the 1.6M-edge list across 8 cores
(200k edges each), replicate atoms/tables. Two device launches:

  Launch 1 (CN): edges sorted by i-atom on host into a padded
    [50048, K] slot matrix per core; device computes the D3
    coordination-number counting function per slot, dense-reduces rows
    to per-atom CN partials, AllReduce-psums CN across the 8 cores, and
    computes the per-atom Gaussian C6-interpolation weights W[50048,5].

  Host: gathers W rows to edge endpoints (index marshalling only).

  Launch 2 (energy): plain per-edge arrays; device computes BJ-damped
    pair energies e = c6_ij * u(d) with c6_ij = Wi^T B Wj (B = gathered
    5x5 C6 block), reduces to per-core partials; host sums partials.

All transcendentals use the {Ln, Exp} activation table set only
(sigmoid via exp, sqrt via exp(0.5 ln)) so there is a single ACT table
load in the whole kernel.
"""

import sys

sys.path.insert(0, "/opt/trn_rl_repo")

import numpy as np

import concourse.bacc as bacc
import concourse.bass as bass
import concourse.mybir as mybir
import concourse.tile as tile
from concourse import bass_utils

F32 = mybir.dt.float32
AX = mybir.AluOpType
ACTF = mybir.ActivationFunctionType

# Our only transcendentals are Ln and Exp. Steer the ACT table-load pass
# to the combined natural_log_exp set so the kernel needs exactly one
# table load instead of thrashing between the ln-only and exp-only sets
# (~2.7us per reload).
_orig_get_tables = bacc.get_activation_tables


def _ln_exp_tables(module_arch):
    tables = dict(_orig_get_tables(module_arch))
    out = {}
    for name, funcs in tables.items():
        if name == "natural_log_exp_and_others":
            out[name] = funcs
        else:
            out[name] = funcs - {ACTF.Ln, ACTF.Exp}
    return out


bacc.get_activation_tables = _ln_exp_tables

# D3 constants
K1 = 16.0
K2 = 4.0 / 3.0
K3 = 4.0
A1, A2, S6, S8 = 0.4, 5.0, 1.0, 0.78
CN_CUTOFF2 = 25.0 * 25.0
DISP_CUTOFF2 = 50.0 * 50.0

N_ATOMS = 50000
NP_ATOMS = 50048  # = 128 * 391
GRID_C = 391
N_EDGES = 1_600_000
N_CORES = 8
E_CORE = N_EDGES // N_CORES  # 200000
NREF = 5

# launch-2 chunking: slots per partition per chunk
L2_C = 200
L2_NCH = 8  # 128*200*8 = 204800 >= 200000
E_PAD2 = 128 * L2_C * L2_NCH

_cache = {}


def _runner(nc, out_names):
    """Compile once, return a callable(in_maps) -> list of out dicts."""
    import jax
    from jax.sharding import Mesh, PartitionSpec
    from jax.experimental.shard_map import shard_map
    from concourse import bass2jax

    bass2jax.install_neuronx_cc_hook()

    partition_name = (
        nc.partition_id_tensor.name if nc.partition_id_tensor else None
    )
    in_names = []
    out_avals = []
    zero_outs = []
    onames = []
    for alloc in nc.m.functions[0].allocations:
        if not isinstance(alloc, mybir.MemoryLocationSet):
            continue
        name = alloc.memorylocations[0].name
        if alloc.kind == "ExternalInput":
            if name != partition_name:
                in_names.append(name)
        elif alloc.kind == "ExternalOutput":
            shape = list(alloc.tensor_shape)
            dt = mybir.dt.np(alloc.dtype)
            onames.append(name)
            out_avals.append(jax.core.ShapedArray(shape, dt))
            zero_outs.append(np.zeros(shape, dt))
    n_params = len(in_names)
    all_in = list(in_names) + list(onames)
    if partition_name is not None:
        all_in.append(partition_name)

    from concourse.bass2jax import _bass_exec_p, partition_id_tensor

    def _body(*args):
        operands = list(args)
        if partition_name is not None:
            operands.append(partition_id_tensor())
        outs = _bass_exec_p.bind(
            *operands,
            out_avals=tuple(out_avals),
            in_names=tuple(all_in),
            out_names=tuple(onames),
            lowering_input_output_aliases=(),
            sim_require_finite=True,
            sim_require_nnan=True,
            nc=nc,
        )
        return tuple(outs)

    devices = jax.devices()[:N_CORES]
    mesh = Mesh(np.asarray(devices), ("core",))
    donate = tuple(range(n_params, n_params + len(onames)))
    sharded = jax.jit(
        shard_map(
            _body,
            mesh=mesh,
            in_specs=(PartitionSpec("core"),) * (n_params + len(onames)),
            out_specs=(PartitionSpec("core"),) * len(onames),
            check_rep=False,
        ),
        donate_argnums=donate,
        keep_unused=True,
    )

    def _concat(in_maps):
        per_core = [[np.asarray(m[n]) for n in in_names] for m in in_maps]
        return [
            np.concatenate([per_core[c][i] for c in range(N_CORES)], axis=0)
            for i in range(n_params)
        ]

    def _zeros():
        return [
            np.zeros((N_CORES * z.shape[0], *z.shape[1:]), z.dtype)
            for z in zero_outs
        ]

    def _unpack(out_arrs):
        return [
            {
                n: np.asarray(out_arrs[i]).reshape(
                    N_CORES, *out_avals[i].shape
                )[c]
                for i, n in enumerate(onames)
            }
            for c in range(N_CORES)
        ]

    def run(in_maps):
        return _unpack(sharded(*_concat(in_maps), *_zeros()))

    def run_timed(in_maps, iters=3):
        """Pre-stage inputs on device, time execute-only. Returns
        (results, best_seconds)."""
        import time
        from jax.sharding import NamedSharding

        sh = NamedSharding(mesh, PartitionSpec("core"))
        staged = [jax.device_put(a, sh) for a in _concat(in_maps)]
        out = sharded(*staged, *_zeros())  # warm
        jax.block_until_ready(out)
        best = float("inf")
        for _ in range(iters):
            z = [jax.device_put(a, sh) for a in _zeros()]
            jax.block_until_ready(z)
            t0 = time.perf_counter()
            out = sharded(*staged, *z)
            jax.block_until_ready(out)
            best = min(best, time.perf_counter() - t0)
        return _unpack(out), best

    run.run_timed = run_timed
    return run


# ---------------------------------------------------------------- launch 1
def _register_consts(nc, values):
    for value in values:
        t = nc.alloc_sbuf_tensor(f"constx-f32-{value}", [128, 1], F32)
        nc.gpsimd.memset(t.ap(), value)
        nc.const_aps.aps[(F32, value)] = t.ap()
    nc.all_engine_barrier()


def build_launch1(K):
    """CN pass: padded slot matrix -> cn grid -> AllReduce -> W.

    k-major layout: pjt[k, atom, 4] (j-side per slot), slf[atom, 4]
    (i-side, constant per atom, read via broadcast APs). Compute runs
    full-width [128, Kc*391] per chunk to amortize DVE instruction
    overhead.
    """
    nc = bacc.Bacc(None, target_bir_lowering=False, num_devices=N_CORES)
    _register_consts(nc, [1e-20, K1])
    pjt = nc.dram_tensor("pjt", [K, NP_ATOMS, 4], F32, kind="ExternalInput")
    slf = nc.dram_tensor("slf", [NP_ATOMS, 4], F32, kind="ExternalInput")
    cnr = nc.dram_tensor("cnr", [NP_ATOMS, NREF], F32, kind="ExternalInput")
    wout = nc.dram_tensor("wout", [NP_ATOMS, NREF], F32, kind="ExternalOutput")
    cnout = nc.dram_tensor("cnout", [128, GRID_C], F32, kind="ExternalOutput")

    KC = 4  # k-slots per chunk
    G = GRID_C

    with tile.TileContext(nc) as tc:
        with (
            tc.tile_pool(name="io", bufs=2) as io,
            tc.tile_pool(name="tmp", bufs=1) as tp,
            tc.tile_pool(name="acc", bufs=1) as ac,
            tc.tile_pool(name="dram", bufs=1, space="DRAM") as dr,
        ):
            sl = ac.tile([128, G * 4], F32)
            nc.sync.dma_start(
                sl[:], slf[:].rearrange("(p c) f -> p (c f)", p=128)
            )
            slv = sl[:].rearrange("p (c f) -> p c f", f=4)

            def selfb(f, kc):
                # [128, G] field -> [128, kc, G] broadcast over k
                return (
                    slv[:, :, f]
                    .to_broadcast([128, G, kc])
                    .rearrange("p c k -> p k c")
                )

            cng = ac.tile([128, GRID_C], F32)
            nc.vector.memset(cng[:], 0.0)
            k0 = 0
            while k0 < K:
                kc = min(KC, K - k0)
                t = io.tile([128, KC * G * 4], F32, tag="pjin")
                for ki in range(kc):
                    nc.sync.dma_start(
                        t[:].rearrange("p (k m) -> p k m", k=KC)[:, ki, :],
                        pjt[k0 + ki].rearrange("(p c) f -> p (c f)", p=128),
                    )
                v = t[:].rearrange("p (k c f) -> p k c f", k=KC, f=4)[:, :kc]
                S = kc * G
                dx = tp.tile([128, KC * G], F32, tag="dx")
                dy = tp.tile([128, KC * G], F32, tag="dy")
                d2 = tp.tile([128, KC * G], F32, tag="d2")
                rr = tp.tile([128, KC * G], F32, tag="rr")
                dxv = dx[:, :S].rearrange("p (k c) -> p k c", k=kc)
                dyv = dy[:, :S].rearrange("p (k c) -> p k c", k=kc)
                d2v = d2[:, :S].rearrange("p (k c) -> p k c", k=kc)
                rrv = rr[:, :S].rearrange("p (k c) -> p k c", k=kc)
                nc.vector.tensor_tensor(dxv, v[:, :, :, 0], selfb(0, kc), op=AX.subtract)
                nc.vector.tensor_tensor(dyv, v[:, :, :, 1], selfb(1, kc), op=AX.subtract)
                nc.vector.tensor_tensor(rrv, v[:, :, :, 3], selfb(3, kc), op=AX.add)
                nc.vector.tensor_tensor(d2[:, :S], dx[:, :S], dx[:, :S], op=AX.mult)
                nc.vector.tensor_tensor(dx[:, :S], dy[:, :S], dy[:, :S], op=AX.mult)
                nc.vector.tensor_tensor(d2[:, :S], d2[:, :S], dx[:, :S], op=AX.add)
                nc.vector.tensor_tensor(dyv, v[:, :, :, 2], selfb(2, kc), op=AX.subtract)
                nc.vector.tensor_tensor(dx[:, :S], dy[:, :S], dy[:, :S], op=AX.mult)
                nc.vector.tensor_tensor(d2[:, :S], d2[:, :S], dx[:, :S], op=AX.add)
                ln_d2 = tp.tile([128, KC * G], F32, tag="lnd2")
                ln_rr = tp.tile([128, KC * G], F32, tag="lnrr")
                nc.scalar.activation(ln_d2[:, :S], d2[:, :S], ACTF.Ln, bias=1e-20)
                nc.scalar.activation(ln_rr[:, :S], rr[:, :S], ACTF.Ln)
                arg = tp.tile([128, KC * G], F32, tag="arg")
                nc.vector.tensor_scalar(arg[:, :S], ln_d2[:, :S], -0.5, None, op0=AX.mult)
                nc.vector.tensor_tensor(arg[:, :S], arg[:, :S], ln_rr[:, :S], op=AX.add)
                tt = tp.tile([128, KC * G], F32, tag="tt")
                nc.scalar.activation(tt[:, :S], arg[:, :S], ACTF.Exp)
                g = tp.tile([128, KC * G], F32, tag="g")
                nc.scalar.activation(g[:, :S], tt[:, :S], ACTF.Exp, bias=K1, scale=-K1 * K2)
                nc.vector.tensor_scalar(g[:, :S], g[:, :S], 1.0, None, op0=AX.add)
                rec = tp.tile([128, KC * G], F32, tag="rec")
                nc.vector.reciprocal(rec[:, :S], g[:, :S])
                msk = tp.tile([128, KC * G], F32, tag="msk")
                nc.vector.tensor_scalar(msk[:, :S], d2[:, :S], CN_CUTOFF2, None, op0=AX.is_lt)
                nc.vector.tensor_tensor(rec[:, :S], rec[:, :S], msk[:, :S], op=AX.mult)
                # reduce over k (strided innermost) and accumulate
                part = tp.tile([128, G], F32, tag="part")
                nc.vector.tensor_reduce(
                    part[:],
                    rec[:, :S]
                    .rearrange("p (k c) -> p k c", k=kc)
                    .rearrange("p k c -> p c k"),
                    axis=mybir.AxisListType.X,
                    op=AX.add,
                )
                nc.vector.tensor_tensor(cng[:], cng[:], part[:], op=AX.add)
                k0 += kc

            # AllReduce cn across cores (psum)
            cin = dr.tile([128, GRID_C], F32)
            cout = dr.tile([128, GRID_C], F32)
            nc.sync.dma_start(cin[:], cng[:])
            nc.gpsimd.collective_compute(
                "AllReduce",
                AX.add,
                replica_groups=[list(range(N_CORES))],
                ins=[cin[:].opt()],
                outs=[cout[:].opt()],
            )
            cn = ac.tile([128, GRID_C], F32)
            nc.sync.dma_start(cn[:], cout[:])
            nc.sync.dma_start(cnout[:], cn[:])

            # ---- W build (per atom) ----
            G = GRID_C
            cr = ac.tile([128, G * NREF], F32)
            nc.sync.dma_start(
                cr[:], cnr[:].rearrange("(p c) r -> p (c r)", p=128)
            )
            crv = cr[:].rearrange("p (c r) -> p c r", r=NREF)
            gw = ac.tile([128, G * NREF], F32)
            gwv = gw[:].rearrange("p (c r) -> p c r", r=NREF)
            mk = ac.tile([128, G * NREF], F32)
            mkv = mk[:].rearrange("p (c r) -> p c r", r=NREF)
            dr_ = tp.tile([128, G], F32, tag="wdr")
            for r in range(NREF):
                nc.vector.tensor_tensor(dr_[:], cn[:], crv[:, :, r], op=AX.subtract)
                nc.vector.tensor_tensor(dr_[:], dr_[:], dr_[:], op=AX.mult)
                nc.scalar.activation(gwv[:, :, r], dr_[:], ACTF.Exp, scale=-K3)
            nc.vector.tensor_scalar(mk[:], cr[:], 0.0, None, op0=AX.is_ge)
            nc.vector.tensor_tensor(gw[:], gw[:], mk[:], op=AX.mult)
            norm = tp.tile([128, G], F32, tag="wnorm")
            nc.vector.tensor_reduce(
                norm[:], gwv[:, :, :], axis=mybir.AxisListType.X, op=AX.add
            )
            # maxv = ref4 if ref4>=0 else ref3
            maxv = tp.tile([128, G], F32, tag="wmaxv")
            t1 = tp.tile([128, G], F32, tag="wt1")
            nc.vector.tensor_tensor(
                maxv[:], crv[:, :, NREF - 1], mkv[:, :, NREF - 1], op=AX.mult
            )
            nc.vector.tensor_scalar(
                t1[:], mkv[:, :, NREF - 1], -1.0, 1.0, op0=AX.mult, op1=AX.add
            )
            nc.vector.tensor_tensor(t1[:], t1[:], crv[:, :, NREF - 2], op=AX.mult)
            nc.vector.tensor_tensor(maxv[:], maxv[:], t1[:], op=AX.add)
            # usefb / denom
            usefb = tp.tile([128, G], F32, tag="wufb")
            nc.vector.tensor_scalar(usefb[:], norm[:], 1e-30, None, op0=AX.is_le)
            nofb = tp.tile([128, G], F32, tag="wnfb")
            nc.vector.tensor_scalar(
                nofb[:], usefb[:], -1.0, 1.0, op0=AX.mult, op1=AX.add
            )
            nc.vector.tensor_scalar(norm[:], norm[:], 1e-30, None, op0=AX.max)
            rn = tp.tile([128, G], F32, tag="wrn")
            nc.vector.reciprocal(rn[:], norm[:])
            nc.vector.tensor_tensor(rn[:], rn[:], nofb[:], op=AX.mult)
            wpack = ac.tile([128, G * NREF], F32)
            wv = wpack[:].rearrange("p (c r) -> p c r", r=NREF)
            fb = tp.tile([128, G], F32, tag="wfb")
            for r in range(NREF):
                nc.vector.tensor_tensor(fb[:], crv[:, :, r], maxv[:], op=AX.is_equal)
                nc.vector.tensor_tensor(fb[:], fb[:], mkv[:, :, r], op=AX.mult)
                nc.vector.tensor_tensor(fb[:], fb[:], usefb[:], op=AX.mult)
                nc.vector.tensor_tensor(
                    wv[:, :, r], gwv[:, :, r], rn[:], op=AX.mult
                )
                nc.vector.tensor_tensor(
                    wv[:, :, r], wv[:, :, r], fb[:], op=AX.add
                )
            nc.sync.dma_start(
                wout[:].rearrange("(p c) r -> p (c r)", p=128), wpack[:]
            )
    nc.finalize()
    return nc


# ---------------------------------------------------------------- launch 2
def build_launch2():
    nc = bacc.Bacc(None, target_bir_lowering=False, num_devices=N_CORES)
    # geo: xi yi zi xj yj zj r4i r4j
    geo = nc.dram_tensor("geo", [E_PAD2, 8], F32, kind="ExternalInput")
    wij = nc.dram_tensor("wij", [E_PAD2, 2 * NREF], F32, kind="ExternalInput")
    c6b = nc.dram_tensor("c6b", [E_PAD2, 25], F32, kind="ExternalInput")
    eout = nc.dram_tensor("eout", [128, 1], F32, kind="ExternalOutput")

    C = L2_C
    with tile.TileContext(nc) as tc:
        with (
            tc.tile_pool(name="io", bufs=2) as io,
            tc.tile_pool(name="tmp", bufs=2) as tp,
            tc.tile_pool(name="acc", bufs=1) as ac,
        ):
            eacc = ac.tile([128, 1], F32)
            nc.vector.memset(eacc[:], 0.0)
            for ch in range(L2_NCH):
                e0 = ch * 128 * C
                g = io.tile([128, C * 8], F32, tag="geo")
                nc.sync.dma_start(
                    g[:],
                    geo[e0 : e0 + 128 * C, :].rearrange(
                        "(p c) f -> p (c f)", p=128
                    ),
                )
                gv = g[:].rearrange("p (c f) -> p c f", f=8)
                w = io.tile([128, C * 2 * NREF], F32, tag="wij")
                nc.sync.dma_start(
                    w[:],
                    wij[e0 : e0 + 128 * C, :].rearrange(
                        "(p c) f -> p (c f)", p=128
                    ),
                )
                wvv = w[:].rearrange("p (c f) -> p c f", f=2 * NREF)
                cb = io.tile([128, C * 25], F32, tag="c6b")
                nc.sync.dma_start(
                    cb[:],
                    c6b[e0 : e0 + 128 * C, :].rearrange(
                        "(p c) f -> p (c f)", p=128
                    ),
                )
                # d2
                dx = tp.tile([128, C], F32, tag="dx")
                dy = tp.tile([128, C], F32, tag="dy")
                d2 = tp.tile([128, C], F32, tag="d2")
                nc.vector.tensor_tensor(dx[:], gv[:, :, 0], gv[:, :, 3], op=AX.subtract)
                nc.vector.tensor_tensor(dy[:], gv[:, :, 1], gv[:, :, 4], op=AX.subtract)
                nc.vector.tensor_tensor(d2[:], dx[:], dx[:], op=AX.mult)
                nc.vector.tensor_tensor(dx[:], dy[:], dy[:], op=AX.mult)
                nc.vector.tensor_tensor(d2[:], d2[:], dx[:], op=AX.add)
                nc.vector.tensor_tensor(dy[:], gv[:, :, 2], gv[:, :, 5], op=AX.subtract)
                nc.vector.tensor_tensor(dx[:], dy[:], dy[:], op=AX.mult)
                nc.vector.tensor_tensor(d2[:], d2[:], dx[:], op=AX.add)
                nc.vector.tensor_scalar(d2[:], d2[:], 1e-20, None, op0=AX.add)
                # q = r4i*r4j ; sq = sqrt(q) = exp(0.5 ln q); f = A1*sqrt(3q)+A2
                q = tp.tile([128, C], F32, tag="q")
                nc.vector.tensor_tensor(q[:], gv[:, :, 6], gv[:, :, 7], op=AX.mult)
                lnq = tp.tile([128, C], F32, tag="lnq")
                nc.scalar.activation(lnq[:], q[:], ACTF.Ln)
                sq = tp.tile([128, C], F32, tag="sq")
                nc.scalar.activation(sq[:], lnq[:], ACTF.Exp, scale=0.5)
                f = tp.tile([128, C], F32, tag="f")
                nc.vector.tensor_scalar(
                    f[:], sq[:], A1 * np.sqrt(3.0), A2, op0=AX.mult, op1=AX.add
                )
                f2 = tp.tile([128, C], F32, tag="f2")
                nc.vector.tensor_tensor(f2[:], f[:], f[:], op=AX.mult)
                f4 = tp.tile([128, C], F32, tag="f4")
                nc.vector.tensor_tensor(f4[:], f2[:], f2[:], op=AX.mult)
                f6 = tp.tile([128, C], F32, tag="f6")
                nc.vector.tensor_tensor(f6[:], f4[:], f2[:], op=AX.mult)
                nc.vector.tensor_tensor(f4[:], f4[:], f4[:], op=AX.mult)  # f8
                d4 = tp.tile([128, C], F32, tag="d4")
                nc.vector.tensor_tensor(d4[:], d2[:], d2[:], op=AX.mult)
                d6 = tp.tile([128, C], F32, tag="d6")
                nc.vector.tensor_tensor(d6[:], d4[:], d2[:], op=AX.mult)
                nc.vector.tensor_tensor(d4[:], d4[:], d4[:], op=AX.mult)  # d8
                nc.vector.tensor_tensor(d6[:], d6[:], f6[:], op=AX.add)
                nc.vector.tensor_tensor(d4[:], d4[:], f4[:], op=AX.add)
                r6 = tp.tile([128, C], F32, tag="r6")
                nc.vector.reciprocal(r6[:], d6[:])
                r8 = tp.tile([128, C], F32, tag="r8")
                nc.vector.reciprocal(r8[:], d4[:])
                # u = (S6*r6 + 3*S8*q*r8) * (d2<2500)   [sign applied at end]
                nc.vector.tensor_tensor(r8[:], r8[:], q[:], op=AX.mult)
                nc.vector.tensor_scalar(r8[:], r8[:], 3.0 * S8, None, op0=AX.mult)
                nc.vector.tensor_scalar(r6[:], r6[:], S6, None, op0=AX.mult)
                nc.vector.tensor_tensor(r6[:], r6[:], r8[:], op=AX.add)
                m50 = tp.tile([128, C], F32, tag="m50")
                nc.vector.tensor_scalar(
                    m50[:], d2[:], DISP_CUTOFF2, None, op0=AX.is_lt
                )
                nc.vector.tensor_tensor(r6[:], r6[:], m50[:], op=AX.mult)
                # einsum: c6 = sum_ab Wi_a Wj_b B_ab
                op = tp.tile([128, C * 25], F32, tag="op")
                opv = op[:].rearrange("p (c a b) -> p c a b", a=NREF, b=NREF)
                wiB = wvv[:, :, 0:NREF].to_broadcast([128, C, NREF, NREF])
                wjB = (
                    wvv[:, :, NREF : 2 * NREF]
                    .to_broadcast([128, C, NREF, NREF])
                    .rearrange("p c b a -> p c a b")
                )
                nc.vector.tensor_tensor(opv, wiB, wjB, op=AX.mult)
                nc.vector.tensor_tensor(op[:], op[:], cb[:], op=AX.mult)
                c6 = tp.tile([128, C], F32, tag="c6")
                nc.vector.tensor_reduce(
                    c6[:],
                    op[:].rearrange("p (c e) -> p c e", e=25),
                    axis=mybir.AxisListType.X,
                    op=AX.add,
                )
                nc.vector.tensor_tensor(c6[:], c6[:], r6[:], op=AX.mult)
                er = tp.tile([128, 1], F32, tag="er")
                nc.vector.tensor_reduce(
                    er[:], c6[:], axis=mybir.AxisListType.X, op=AX.add
                )
                nc.vector.tensor_tensor(eacc[:], eacc[:], er[:], op=AX.add)
            nc.vector.tensor_scalar(eacc[:], eacc[:], -0.5, None, op0=AX.mult)
            nc.sync.dma_start(eout[:], eacc[:])
    nc.finalize()
    return nc


# ---------------------------------------------------------------- host side
def _prep(positions, numbers, edges_i, edges_j, rcov, r4r2, c6_table, cn_ref):
    """Host-side sharding + index marshalling. Returns (K, l1_maps, meta)."""
    pos = np.zeros((NP_ATOMS, 3), np.float32)
    pos[:N_ATOMS] = positions
    # pad atoms far away so any accidental reference is masked out
    pos[N_ATOMS:] = 1.0e4
    num = np.zeros(NP_ATOMS, np.int32)
    num[:N_ATOMS] = numbers
    rcov_a = rcov[num].astype(np.float32)
    r4r2_a = r4r2[num].astype(np.float32)
    cnr_a = cn_ref[num].astype(np.float32)  # [NP, 5]

    cores = []
    Kmax = 1
    for c in range(N_CORES):
        ei = edges_i[c * E_CORE : (c + 1) * E_CORE].astype(np.int64)
        ej = edges_j[c * E_CORE : (c + 1) * E_CORE].astype(np.int64)
        order = np.argsort(ei, kind="stable")
        ei, ej = ei[order], ej[order]
        counts = np.bincount(ei, minlength=NP_ATOMS)
        Kmax = max(Kmax, int(counts.max()))
        cores.append((ei, ej, counts))
    K = int(Kmax)

    l1_maps = []
    metas = []
    for c in range(N_CORES):
        ei, ej, counts = cores[c]
        starts = np.zeros(NP_ATOMS, np.int64)
        starts[1:] = np.cumsum(counts)[:-1]
        kpos = np.arange(E_CORE, dtype=np.int64) - starts[ei]
        # k-major j-side slots [K, NP, 4]; pad xj=1e3 (masked), rcov=0.5
        pjt = np.zeros((K, NP_ATOMS, 4), np.float32)
        pjt[:, :, 0] = 1.0e3
        pjt[:, :, 3] = 0.5
        pjt[kpos, ei, 0:3] = pos[ej]
        pjt[kpos, ei, 3] = rcov_a[ej]
        slfa = np.zeros((NP_ATOMS, 4), np.float32)
        slfa[:, 0:3] = pos
        slfa[:, 3] = rcov_a
        l1_maps.append(dict(pjt=pjt, slf=slfa, cnr=cnr_a))
        metas.append((ei, ej))
    return K, l1_maps, metas


def kernel(positions, numbers, edges_i, edges_j, rcov, r4r2, c6_table,
           cn_ref, _times=None):
    K, l1_maps, metas = _prep(
        positions, numbers, edges_i, edges_j, rcov, r4r2, c6_table, cn_ref
    )

    if ("l1", K) not in _cache:
        _cache[("l1", K)] = _runner(build_launch1(K), ["wout", "cnout"])
    run1 = _cache[("l1", K)]
    if _times is not None:
        res1, t1 = run1.run_timed(l1_maps)
        _times.append(t1)
    else:
        res1 = run1(l1_maps)
    W = res1[0]["wout"]  # [NP_ATOMS, 5] (identical on all cores)

    num = np.zeros(NP_ATOMS, np.int32)
    num[:N_ATOMS] = numbers
    pos = np.zeros((NP_ATOMS, 3), np.float32)
    pos[:N_ATOMS] = positions
    r4r2_a = r4r2[num].astype(np.float32)
    c6f = np.ascontiguousarray(c6_table.reshape(95 * 95, 25).astype(np.float32))

    l2_maps = []
    for c in range(N_CORES):
        ei, ej = metas[c]
        geo = np.zeros((E_PAD2, 8), np.float32)
        geo[:, 3] = 1.0e3  # pad: far apart -> masked
        geo[:, 6:8] = 1.0  # pad: ln(1)=0 safe
        geo[:E_CORE, 0:3] = pos[ei]
        geo[:E_CORE, 3:6] = pos[ej]
        geo[:E_CORE, 6] = r4r2_a[ei]
        geo[:E_CORE, 7] = r4r2_a[ej]
        wij = np.zeros((E_PAD2, 10), np.float32)
        wij[:E_CORE, 0:5] = W[ei]
        wij[:E_CORE, 5:10] = W[ej]
        c6b = np.zeros((E_PAD2, 25), np.float32)
        pair = num[ei].astype(np.int64) * 95 + num[ej]
        c6b[:E_CORE] = c6f[pair]
        l2_maps.append(dict(geo=geo, wij=wij, c6b=c6b))

    if "l2" not in _cache:
        _cache["l2"] = _runner(build_launch2(), ["eout"])
    run2 = _cache["l2"]
    if _times is not None:
        res2, t2 = run2.run_timed(l2_maps)
        _times.append(t2)
    else:
        res2 = run2(l2_maps)
    total = sum(float(res2[c]["eout"].sum()) for c in range(N_CORES))
    return np.float32(total)
